# revision 40
# baseline (speedup 1.0000x reference)
"""Trainium2 Bass kernel for nn_AutoencoderHybrid (12-qubit QAE hybrid).

Math: the circuit measures Z on wires 0..3 only. The CNOT chain (i -> i+1)
propagates information forward only, so each observable Z_w pulled back
through the 2-layer circuit is supported on wires 0..w+1 (at most 0..4).
With the product input state and the diagonal phase fold, each latent is a
REAL quadratic form over the prefix product vector of dim d_w = 2^(w+2):

    latent_w(b) = v_w^T Stil_w v_w,  v_w = kron_{j<=w+1} [cos(x_j/2), sin(x_j/2)]

The prefix vectors A(4)=v_0, B(8)=v_1, D(16)=v_2, R(32)=v_3 all fall out of
one kron tree, concatenated into vcat (60 slots/group, 64-padded, GROUP-MINOR
layout col = 4*slot + g so every DVE op is packed for fp16 2x mode).

Fully transposed dataflow after the kron (no reductions, no lat tile):
  vcatT   (PE transpose, fp16)
  YT      = mprojT^T @ vcatT        (PE, feature-space quadform halves)
  PmT     = YT * vcatT              (DVE, partition-aligned elementwise)
  hT      = W1X0^T @ PmT0 + W1X1^T @ PmT1   (PE, accumulate; the grouped
            reduction AND the W1 layer folded into one constant matrix)
  hT_s    = relu(hT + b1)           (ACT, bias per-partition)
  y4      = hT_s^T @ w2blk          (PE)
  y       = y4 + b2                 (DVE add doubles as PSUM->SBUF copy)

All constants (mproj/W1X/w2 fp16, b1/b2 f32) are host-prepared; batch work
(B = 8192) runs on 8 NeuronCores data parallel, 1024 rows/core as b = 8p+c.

Scheduling: x DMA hoisted before the entry all-engine barrier; constants in
two packed DMAs behind it on SP; per-half output DMAs issued as ready.
"""
import math
import numpy as np

N5 = 5
NLAYERS = 2
LATENT = 4
B = 8192
NCORES = 8
BLOC = B // NCORES  # 1024

# slot layout per group: [R:0..32 | D:32..48 | B:48..56 | A:56..60 | pad]
_SLOT_OFF = {3: 0, 2: 32, 1: 48, 0: 56}
_SLOT_DIM = {3: 32, 2: 16, 1: 8, 0: 4}

# ----------------------------------------------------------------------------
# Host-side constant construction (pure numpy)
# ----------------------------------------------------------------------------


def _rot(phi, theta, omega):
    c, s = np.cos(theta / 2), np.sin(theta / 2)
    ep = np.exp(-0.5j * (phi + omega))
    em = np.exp(-0.5j * (phi - omega))
    return np.array([[ep * c, -np.conj(em) * s], [em * s, np.conj(ep) * c]],
                    dtype=np.complex128)


def _build_S(q_params):
    """(4, 32, 32) real symmetric: latent_w = r^T S_w r (unit-norm slots)."""
    qp = np.asarray(q_params, np.float64)
    dim = 2 ** N5
    eye2 = np.eye(2)

    def kron_at(U, wire):
        M = np.array([[1.0]])
        for j in range(N5):
            M = np.kron(M, U if j == wire else eye2)
        return M

    def cnot_mat(c, t):
        M = np.zeros((dim, dim))
        for z in range(dim):
            bits = [(z >> (N5 - 1 - j)) & 1 for j in range(N5)]
            if bits[c] == 1:
                bits[t] ^= 1
            z2 = 0
            for b in bits:
                z2 = (z2 << 1) | b
            M[z2, z] = 1.0
        return M

    V = np.eye(dim, dtype=np.complex128)
    for l in range(NLAYERS):
        for i in range(N5):
            V = kron_at(_rot(*qp[l, i]), i) @ V
        for i in range(N5 - 1):
            V = cnot_mat(i, i + 1) @ V

    pc = np.array([bin(z).count("1") for z in range(dim)])
    D = np.diag((-1j) ** pc)
    VD = V @ D
    Ss = []
    for w in range(LATENT):
        zdiag = np.array([1.0 if ((z >> (N5 - 1 - w)) & 1) == 0 else -1.0
                          for z in range(dim)])
        O = VD.conj().T @ (zdiag[:, None] * VD)
        Ss.append(np.real(O))
    return np.stack(Ss)


def _host_consts(q_params, W1, b1, W2, b2):
    S = _build_S(q_params)
    W1 = np.asarray(W1, np.float64)
    b1 = np.asarray(b1, np.float64)
    W2 = np.asarray(W2, np.float64)
    b2 = np.asarray(b2, np.float64)

    def stil(w):
        d = _SLOT_DIM[w]
        tail = 32 // d
        # device slots carry cos/2 where bit=0 and UNSCALED sin where bit=1:
        # compensate 2^(#zero-bits) per side
        St = S[w][::tail, ::tail].copy()
        nb = int(round(math.log2(d)))
        zc = np.array([nb - bin(i).count("1") for i in range(d)], np.float64)
        return St * (2.0 ** zc)[:, None] * (2.0 ** zc)[None, :]

    # mproj chunk0: rows (4i+g) i<32 (R slots) -> w3 outs (4i'+g)
    mp0 = np.zeros((128, 128))
    St3 = stil(3)
    for g in range(4):
        mp0[g::4, g::4] = St3.T
    # chunk1: rows (4s+g), s = slot-32 ([D|B|A|pad]) -> oslots 32..60
    # (padded to 128 cols of zeros so YT1 fills all 128 partitions)
    mp1 = np.zeros((128, 128))
    for w, so in ((2, 0), (1, 16), (0, 24)):
        Sw = stil(w)
        d = _SLOT_DIM[w]
        for g in range(4):
            mp1[4 * so + g:4 * (so + d) + g:4,
                4 * so + g:4 * (so + d) + g:4] = Sw.T

    # W1X chunks fold the grouped reduction + W1: rows k -> (slot, g),
    # cols (32g + a)
    def w1x_chunk(c, rows):
        M = np.zeros((rows, 128))
        for k in range(rows):
            slot = (128 * c + k) >> 2
            g = k & 3
            if slot < 32:
                w = 3
            elif slot < 48:
                w = 2
            elif slot < 56:
                w = 1
            elif slot < 60:
                w = 0
            else:
                continue
            M[k, 32 * g:32 * g + 32] = W1[:, w]
        return M
    W1X0 = w1x_chunk(0, 128)
    W1X1 = w1x_chunk(1, 128)  # rows 112.. are zero (pad slots)

    w2blk = np.zeros((128, 48))
    for g in range(4):
        w2blk[32 * g:32 * g + 32, 12 * g:12 * g + 12] = W2.T

    # fp16 packed consts: [mp0 | mp1 | W1X0 | W1X1 | w2]
    hcst = np.zeros((128, 560), np.float16)
    hcst[:, 0:128] = mp0
    hcst[:, 128:256] = mp1
    hcst[:, 256:384] = W1X0
    hcst[:, 384:512] = W1X1
    hcst[:, 512:560] = w2blk

    # f32 consts: [b2rep | b1T]
    fcst = np.zeros((128, 52), np.float32)
    fcst[:, 0:48] = np.tile(b2, 4)[None, :]
    fcst[:, 48] = np.tile(b1, 4)
    return dict(hcst=hcst, fcst=fcst)


# ----------------------------------------------------------------------------
# Device kernel body (Bass/Tile)
# ----------------------------------------------------------------------------


def _build_body(ctx, tc, x, hcst, fcst, y):
    import concourse.bass as bass
    from concourse import mybir
    nc = tc.nc
    f32 = mybir.dt.float32
    f16 = mybir.dt.float16
    AF = mybir.ActivationFunctionType
    ALU = mybir.AluOpType

    def fv(t, col, dims):
        """View of tile t at free-offset col with custom free dims."""
        return bass.AP(tensor=t.tensor, offset=t.offset + col,
                       ap=[list(t.ap[0])] + [list(d) for d in dims])

    consts = ctx.enter_context(tc.tile_pool(name="consts", bufs=1))
    sb = ctx.enter_context(tc.tile_pool(name="sb", bufs=1))
    sb2 = ctx.enter_context(tc.tile_pool(name="sb2", bufs=2))
    ps = ctx.enter_context(tc.tile_pool(name="ps", bufs=2, space="PSUM"))

    # ---- x load (fp16, host-converted): hoisted before the entry barrier
    x_s = sb.tile([128, 96], f16)
    xa = bass.AP(tensor=x.tensor, offset=0, ap=[[96, 128], [1, 96]])
    xdma = nc.sync.dma_start(x_s[:, :], xa)

    # ---- packed constants, also SP queue (issue right behind x)
    h_s = consts.tile([128, 560], f16)
    nc.sync.dma_start(h_s[:, :], hcst)
    f_s = consts.tile([128, 52], f32)
    nc.sync.dma_start(f_s[:, :], fcst)
    mp0_v = h_s[:, 0:128]
    mp1_v = h_s[:, 128:256]
    w1x0_v = h_s[:, 256:384]
    w1x1_v = h_s[:, 384:512]
    w2_v = h_s[:, 512:560]
    b2_v = f_s[:, 0:48]
    b1_v = f_s[:, 48:49]

    bias_c = consts.tile([128, 1], f32)
    nc.vector.memset(bias_c[:, :], math.pi / 2)
    # fp16 identity for the transposes, built on Pool (idle at start)
    ones16 = consts.tile([128, 128], f16)
    nc.gpsimd.memset(ones16[:, :], 1.0)
    ident = consts.tile([128, 128], f16)
    nc.gpsimd.affine_select(out=ident[:, :], in_=ones16[:, :],
                            pattern=[[1, 128]],
                            compare_op=ALU.is_equal, fill=0.0,
                            base=0, channel_multiplier=-1)

    # warm the ACT Sin table immediately (single-dep activation keeps the
    # auto-inserted LoadActFuncSet ahead of any multi-wait semaphore bundle)
    warm = consts.tile([128, 1], f32)
    nc.scalar.activation(warm[:, :], bias_c[:, 0:1], AF.Sin,
                         bias=0.0, scale=1.0)

    # vcat tiles + zero their pad slots early (pads flow into the transposes)
    vch = []
    for h in (0, 1):
        vc = sb.tile([128, 256], f16, name=f"vc{h}")
        E = nc.vector if h == 0 else nc.gpsimd
        E.memset(vc[:, 240:256], 0.0)
        vch.append(vc)

    # ---- trig: cs[40h + 20t + 4j + g]: t=0: cos(x_j/2)/2 = c4^2 - 1/2,
    # t=1: sin(x_j/2) DIRECT from the table (|x/2| < pi; the mixed per-slot
    # scaling is folded into mproj on the host). c4 = cos(x/4) first.
    sc4 = sb.tile([128, 40], f16)
    cs = sb.tile([128, 80], f16)
    nc.scalar.activation(fv(sc4, 0, [[20, 2], [4, 5], [1, 4]]),
                         fv(x_s, 0, [[48, 2], [1, 5], [12, 4]]),
                         AF.Sin, bias=bias_c[:, 0:1], scale=0.25)
    nc.scalar.activation(fv(cs, 20, [[40, 2], [4, 5], [1, 4]]),
                         fv(x_s, 0, [[48, 2], [1, 5], [12, 4]]),
                         AF.Sin, bias=0.0, scale=0.5)

    # ---- per-half slot assembly + prefix kron tree into vcat (all packed)
    from concourse.bass import _add_dep_helper
    kron_last = {}
    for h in (0, 1):
        E = nc.vector if h == 0 else nc.gpsimd
        sqt = sb.tile([128, 20], f16, name=f"sq{h}")
        ctmp = sb.tile([128, 16], f16, name=f"ct{h}")
        vc = vch[h]

        class _CView:
            """cs columns for this half (offset 40h)."""
            tensor = cs.tensor
            offset = cs.offset + 40 * h
            ap = cs.ap
        cst = _CView
        # sq = c4^2 ; cs_c = sq - 1/2  (cs_s came straight from ACT)
        E.tensor_mul(fv(sqt, 0, [[4, 5], [1, 4]]),
                     fv(sc4, 20 * h, [[4, 5], [1, 4]]),
                     fv(sc4, 20 * h, [[4, 5], [1, 4]]))
        E.tensor_scalar_sub(fv(cst, 0, [[4, 5], [1, 4]]),
                            fv(sqt, 0, [[4, 5], [1, 4]]), 0.5)
        # u_j[t, g] at cst col 20t + 4j + g
        # A[a=(z0 z1)] -> vcat slots 56..60 (cols 224..240)
        E.tensor_mul(fv(vc, 224, [[8, 2], [4, 2], [1, 4]]),
                     fv(cst, 4, [[0, 2], [20, 2], [1, 4]]),
                     fv(cst, 0, [[20, 2], [0, 2], [1, 4]]))
        # C[(z3 z4)] -> ctmp
        E.tensor_mul(fv(ctmp, 0, [[8, 2], [4, 2], [1, 4]]),
                     fv(cst, 16, [[0, 2], [20, 2], [1, 4]]),
                     fv(cst, 12, [[20, 2], [0, 2], [1, 4]]))
        # B[(a z2)] -> slots 48..56 (cols 192..224)
        E.tensor_mul(fv(vc, 192, [[8, 4], [4, 2], [1, 4]]),
                     fv(vc, 224, [[4, 4], [0, 2], [1, 4]]),
                     fv(cst, 8, [[0, 4], [20, 2], [1, 4]]))
        # R[(b c2)] -> slots 0..32 (cols 0..128)   (chunk0 = pure R)
        # the big R op runs on DVE for BOTH halves (Pool is 2.4x slower and
        # R gates the chunk0 transpose); D stays on E (parallel, chunk1 only)
        r_i = nc.vector.tensor_mul(fv(vc, 0, [[16, 8], [4, 4], [1, 4]]),
                                   fv(vc, 192, [[4, 8], [0, 4], [1, 4]]),
                                   fv(ctmp, 0, [[0, 8], [4, 4], [1, 4]]))
        # D[(b z3)] -> slots 32..48 (cols 128..192)  (feeds only chunk1)
        d_i = E.tensor_mul(fv(vc, 128, [[8, 8], [4, 2], [1, 4]]),
                           fv(vc, 192, [[4, 8], [0, 2], [1, 4]]),
                           fv(cst, 12, [[0, 8], [20, 2], [1, 4]]))
        kron_last[h] = (r_i, d_i)
    # keep h1's DVE-side R op behind h0's whole kron in the DVE queue
    # (otherwise its Pool dependency head-of-line blocks the h0 chain)
    _add_dep_helper(kron_last[1][0].ins, kron_last[0][1].ins, sync=False,
                    reason="R1 after kron0")

    # ---- all four transposes first (explicit PE order: T's ahead of YT
    # matmuls, so no YT blocks a later transpose's dependents)
    tps = {}
    t_insts = []
    for h in (0, 1):
        for c in (0, 1):
            tp = ps.tile([128, 128], f16, tag="tp", bufs=3)
            ti = nc.tensor.transpose(tp[:, :],
                                     vch[h][:, 128 * c:128 * c + 128],
                                     ident[:, :])
            tps[(h, c)] = tp
            t_insts.append(ti)

    y4_p = ps.tile([128, 96], f32, tag="y4", bufs=1)
    y_s = sb.tile([128, 96], f32)
    for h in (0, 1):
        # both chunks' transposed features in ONE tile -> one merged pm op
        vs = sb2.tile([128, 256], f16, tag=f"vT{h}", bufs=1)
        nc.vector.tensor_copy(vs[:, 0:128], tps[(h, 0)][:, :])
        nc.vector.tensor_copy(vs[:, 128:256], tps[(h, 1)][:, :])
        # YT = mproj^T @ vcatT (per-half PSUM bank holds both chunks; mp1 is
        # zero-padded to 128 cols so the whole bank is written)
        ytb = ps.tile([128, 256], f32, tag=f"YT{h}", bufs=1)
        yt_i = nc.tensor.matmul(ytb[:, 0:128], lhsT=mp0_v, rhs=vs[:, 0:128],
                                start=True, stop=True)
        if h == 0:
            # keep the last transpose ahead of the first YT in the PE queue
            _add_dep_helper(yt_i.ins, t_insts[-1].ins, sync=False,
                            reason="T before YT")
        nc.tensor.matmul(ytb[:, 128:256], lhsT=mp1_v, rhs=vs[:, 128:256],
                         start=True, stop=True)
        # PmT = YT * vcatT: ONE merged elementwise op per half (DVE)
        pm = sb2.tile([128, 256], f16, tag=f"Pm{h}", bufs=1)
        pm_i = nc.vector.tensor_mul(pm[:, :], ytb[:, :], vs[:, :])
        if h == 0:
            pm_h0 = pm_i
        else:
            # h1's PmT mul behind h0's in the DVE queue
            _add_dep_helper(pm_i.ins, pm_h0.ins, sync=False,
                            reason="pm order")
        # hT = W1X0^T @ PmT0 + W1X1^T @ PmT1 (reduction + W1 in one step)
        hT_p = ps.tile([128, 128], f32, tag=f"hT{h}", bufs=1)
        nc.tensor.matmul(hT_p[:, :], lhsT=w1x0_v, rhs=pm[:, 0:128],
                         start=True, stop=False)
        nc.tensor.matmul(hT_p[:, :], lhsT=w1x1_v, rhs=pm[:, 128:256],
                         start=False, stop=True)
        # relu with b1 folded in: h0 on ACT, h1 on DVE (free after the pm
        # muls; saves the ACT 185-cycle access on the critical h1 lane)
        hT_s = sb2.tile([128, 128], f16, tag=f"hTs{h}", bufs=1)
        if h == 0:
            nc.scalar.activation(hT_s[:, :], hT_p[:, :], AF.Relu,
                                 bias=b1_v, scale=1.0)
        else:
            r1_i = nc.vector.tensor_scalar(hT_s[:, :], hT_p[:, :], b1_v,
                                           0.0, ALU.add, ALU.max)
            _add_dep_helper(r1_i.ins, pm_i.ins, sync=False,
                            reason="relu1 after pm1")
        nc.tensor.matmul(y4_p[:, 48 * h:48 * h + 48], lhsT=hT_s[:, :],
                         rhs=w2_v, start=True, stop=True)
        # b2 add doubles as the PSUM->SBUF copy
        nc.vector.tensor_add(fv(y_s, 48 * h, [[1, 48]]),
                             fv(y4_p, 48 * h, [[1, 48]]), b2_v)
        ya = bass.AP(tensor=y.tensor, offset=48 * h, ap=[[96, 128], [1, 48]])
        nc.sync.dma_start(ya, y_s[:, 48 * h:48 * h + 48])

    return xdma


def _hoist_pre_barrier(nc, inst):
    """Move `inst` (a BassInstruction) into the entry block before the first
    SP-engine instruction (i.e. before the all-engine start barrier)."""
    from concourse import mybir
    ins = inst.ins
    fn = nc.m.functions[0]
    blocks = fn.blocks
    src = None
    for b in blocks:
        for i2 in b.instructions:
            if i2.name == ins.name:
                src = b
                break
        if src is not None:
            break
    assert src is not None, "hoist: dma instruction not found"
    entry = blocks[0]
    src.instructions.remove(ins)
    idx = 0
    for k, i2 in enumerate(entry.instructions):
        if i2.engine == mybir.EngineType.SP:
            idx = k
            break
    entry.instructions.insert(idx, ins)


_NC_CACHE = {}


def _get_nc():
    if "nc" in _NC_CACHE:
        return _NC_CACHE["nc"]
    from contextlib import ExitStack
    import concourse.bacc as bacc
    import concourse.tile as tile
    from concourse import mybir
    f32 = mybir.dt.float32
    f16 = mybir.dt.float16
    nc = bacc.Bacc("TRN2", target_bir_lowering=False, debug=False)
    x = nc.dram_tensor("x", [BLOC, 12], f16, kind="ExternalInput").ap()
    hcst = nc.dram_tensor("hcst", [128, 560], f16, kind="ExternalInput").ap()
    fcst = nc.dram_tensor("fcst", [128, 52], f32, kind="ExternalInput").ap()
    y = nc.dram_tensor("y", [BLOC, 12], f32, kind="ExternalOutput").ap()
    with tile.TileContext(nc) as tc:
        with ExitStack() as ctx:
            xdma = _build_body(ctx, tc, x, hcst, fcst, y)
    _hoist_pre_barrier(nc, xdma)
    nc.compile()
    _NC_CACHE["nc"] = nc
    return nc


def _run(inputs_np, consts, trace=False):
    from concourse.bass_utils import run_bass_kernel_spmd
    nc = _get_nc()
    x = np.ascontiguousarray(np.asarray(inputs_np, np.float32).astype(np.float16))
    in_maps = []
    for c in range(NCORES):
        m = {"x": np.ascontiguousarray(x[BLOC * c:BLOC * (c + 1)])}
        m.update(consts)
        in_maps.append(m)
    res = run_bass_kernel_spmd(nc, in_maps, core_ids=list(range(NCORES)),
                               trace=trace)
    out = np.concatenate([r["y"] for r in res.results], axis=0)
    return out.astype(np.float32), res


def kernel(inputs, q_params, W1, b1, W2, b2):
    consts = _host_consts(q_params, W1, b1, W2, b2)
    out, _ = _run(inputs, consts, trace=False)
    return out


# revision 44
# speedup vs baseline: 1.0109x; 1.0109x over previous
"""Trainium2 Bass kernel for nn_AutoencoderHybrid (12-qubit QAE hybrid).

Math: the circuit measures Z on wires 0..3 only. The CNOT chain (i -> i+1)
propagates information forward only, so each observable Z_w pulled back
through the 2-layer circuit is supported on wires 0..w+1 (at most 0..4).
With the product input state and the diagonal phase fold, each latent is a
REAL quadratic form over the prefix product vector of dim d_w = 2^(w+2):

    latent_w(b) = v_w^T Stil_w v_w,  v_w = kron_{j<=w+1} [cos(x_j/2), sin(x_j/2)]

The prefix vectors A(4)=v_0, B(8)=v_1, D(16)=v_2, R(32)=v_3 all fall out of
one kron tree, concatenated into vcat (60 slots/group, 64-padded, GROUP-MINOR
layout col = 4*slot + g so every DVE op is packed for fp16 2x mode).

Fully transposed dataflow after the kron (no reductions, no lat tile):
  vcatT   (PE transpose, fp16)
  YT      = mprojT^T @ vcatT        (PE, feature-space quadform halves)
  PmT     = YT * vcatT              (DVE, partition-aligned elementwise)
  hT      = W1X0^T @ PmT0 + W1X1^T @ PmT1   (PE, accumulate; the grouped
            reduction AND the W1 layer folded into one constant matrix)
  hT_s    = relu(hT + b1)           (ACT, bias per-partition)
  y4      = hT_s^T @ w2blk          (PE)
  y       = y4 + b2                 (DVE add doubles as PSUM->SBUF copy)

All constants (mproj/W1X/w2 fp16, b1/b2 f32) are host-prepared; batch work
(B = 8192) runs on 8 NeuronCores data parallel, 1024 rows/core as b = 8p+c.

Scheduling: x DMA hoisted before the entry all-engine barrier; constants in
two packed DMAs behind it on SP; per-half output DMAs issued as ready.
"""
import math
import numpy as np

N5 = 5
NLAYERS = 2
LATENT = 4
B = 8192
NCORES = 8
BLOC = B // NCORES  # 1024

# slot layout per group: [R:0..32 | D:32..48 | B:48..56 | A:56..60 | pad]
_SLOT_OFF = {3: 0, 2: 32, 1: 48, 0: 56}
_SLOT_DIM = {3: 32, 2: 16, 1: 8, 0: 4}

# ----------------------------------------------------------------------------
# Host-side constant construction (pure numpy)
# ----------------------------------------------------------------------------


def _rot(phi, theta, omega):
    c, s = np.cos(theta / 2), np.sin(theta / 2)
    ep = np.exp(-0.5j * (phi + omega))
    em = np.exp(-0.5j * (phi - omega))
    return np.array([[ep * c, -np.conj(em) * s], [em * s, np.conj(ep) * c]],
                    dtype=np.complex128)


def _build_S(q_params):
    """(4, 32, 32) real symmetric: latent_w = r^T S_w r (unit-norm slots)."""
    qp = np.asarray(q_params, np.float64)
    dim = 2 ** N5
    eye2 = np.eye(2)

    def kron_at(U, wire):
        M = np.array([[1.0]])
        for j in range(N5):
            M = np.kron(M, U if j == wire else eye2)
        return M

    def cnot_mat(c, t):
        M = np.zeros((dim, dim))
        for z in range(dim):
            bits = [(z >> (N5 - 1 - j)) & 1 for j in range(N5)]
            if bits[c] == 1:
                bits[t] ^= 1
            z2 = 0
            for b in bits:
                z2 = (z2 << 1) | b
            M[z2, z] = 1.0
        return M

    V = np.eye(dim, dtype=np.complex128)
    for l in range(NLAYERS):
        for i in range(N5):
            V = kron_at(_rot(*qp[l, i]), i) @ V
        for i in range(N5 - 1):
            V = cnot_mat(i, i + 1) @ V

    pc = np.array([bin(z).count("1") for z in range(dim)])
    D = np.diag((-1j) ** pc)
    VD = V @ D
    Ss = []
    for w in range(LATENT):
        zdiag = np.array([1.0 if ((z >> (N5 - 1 - w)) & 1) == 0 else -1.0
                          for z in range(dim)])
        O = VD.conj().T @ (zdiag[:, None] * VD)
        Ss.append(np.real(O))
    return np.stack(Ss)


def _host_consts(q_params, W1, b1, W2, b2):
    S = _build_S(q_params)
    W1 = np.asarray(W1, np.float64)
    b1 = np.asarray(b1, np.float64)
    W2 = np.asarray(W2, np.float64)
    b2 = np.asarray(b2, np.float64)

    def stil(w):
        d = _SLOT_DIM[w]
        tail = 32 // d
        # device slots carry cos/2 where bit=0 and UNSCALED sin where bit=1:
        # compensate 2^(#zero-bits) per side
        St = S[w][::tail, ::tail].copy()
        nb = int(round(math.log2(d)))
        zc = np.array([nb - bin(i).count("1") for i in range(d)], np.float64)
        return St * (2.0 ** zc)[:, None] * (2.0 ** zc)[None, :]

    # mproj chunk0: rows (4i+g) i<32 (R slots) -> w3 outs (4i'+g)
    mp0 = np.zeros((128, 128))
    St3 = stil(3)
    for g in range(4):
        mp0[g::4, g::4] = St3.T
    # chunk1: rows (4s+g), s = slot-32 ([D|B|A|pad]) -> oslots 32..60
    # (padded to 128 cols of zeros so YT1 fills all 128 partitions)
    mp1 = np.zeros((128, 128))
    for w, so in ((2, 0), (1, 16), (0, 24)):
        Sw = stil(w)
        d = _SLOT_DIM[w]
        for g in range(4):
            mp1[4 * so + g:4 * (so + d) + g:4,
                4 * so + g:4 * (so + d) + g:4] = Sw.T

    # W1X chunks fold the grouped reduction + W1: rows k -> (slot, g),
    # cols (32g + a)
    def w1x_chunk(c, rows):
        M = np.zeros((rows, 128))
        for k in range(rows):
            slot = (128 * c + k) >> 2
            g = k & 3
            if slot < 32:
                w = 3
            elif slot < 48:
                w = 2
            elif slot < 56:
                w = 1
            elif slot < 60:
                w = 0
            else:
                continue
            M[k, 32 * g:32 * g + 32] = W1[:, w]
        return M
    W1X0 = w1x_chunk(0, 128)
    W1X1 = w1x_chunk(1, 128)  # rows 112.. are zero (pad slots)

    w2blk = np.zeros((128, 48))
    for g in range(4):
        w2blk[32 * g:32 * g + 32, 12 * g:12 * g + 12] = W2.T

    # fp16 packed consts: [mp0 | mp1 | W1X0 | W1X1 | w2]
    hcst = np.zeros((128, 560), np.float16)
    hcst[:, 0:128] = mp0
    hcst[:, 128:256] = mp1
    hcst[:, 256:384] = W1X0
    hcst[:, 384:512] = W1X1
    hcst[:, 512:560] = w2blk

    # f32 consts: [b2rep | b1T]
    fcst = np.zeros((128, 52), np.float32)
    fcst[:, 0:48] = np.tile(b2, 4)[None, :]
    fcst[:, 48] = np.tile(b1, 4)
    return dict(hcst=hcst, fcst=fcst)


# ----------------------------------------------------------------------------
# Device kernel body (Bass/Tile)
# ----------------------------------------------------------------------------


def _build_body(ctx, tc, x, hcst, fcst, y):
    import concourse.bass as bass
    from concourse import mybir
    nc = tc.nc
    f32 = mybir.dt.float32
    f16 = mybir.dt.float16
    AF = mybir.ActivationFunctionType
    ALU = mybir.AluOpType

    def fv(t, col, dims):
        """View of tile t at free-offset col with custom free dims."""
        return bass.AP(tensor=t.tensor, offset=t.offset + col,
                       ap=[list(t.ap[0])] + [list(d) for d in dims])

    consts = ctx.enter_context(tc.tile_pool(name="consts", bufs=1))
    sb = ctx.enter_context(tc.tile_pool(name="sb", bufs=1))
    sb2 = ctx.enter_context(tc.tile_pool(name="sb2", bufs=2))
    ps = ctx.enter_context(tc.tile_pool(name="ps", bufs=2, space="PSUM"))

    # ---- x load (fp16, host-converted): hoisted before the entry barrier
    x_s = sb.tile([128, 96], f16)
    xa = bass.AP(tensor=x.tensor, offset=0, ap=[[96, 128], [1, 96]])
    xdma = nc.sync.dma_start(x_s[:, :], xa)

    # ---- packed constants, also SP queue (issue right behind x)
    h_s = consts.tile([128, 560], f16)
    nc.sync.dma_start(h_s[:, :], hcst)
    f_s = consts.tile([128, 52], f32)
    nc.sync.dma_start(f_s[:, :], fcst)
    mp0_v = h_s[:, 0:128]
    mp1_v = h_s[:, 128:256]
    w1x0_v = h_s[:, 256:384]
    w1x1_v = h_s[:, 384:512]
    w2_v = h_s[:, 512:560]
    b2_v = f_s[:, 0:48]
    b1_v = f_s[:, 48:49]

    bias_c = consts.tile([128, 1], f32)
    nc.vector.memset(bias_c[:, :], math.pi / 2)
    # fp16 identity for the transposes, built on Pool (idle at start)
    ones16 = consts.tile([128, 128], f16)
    nc.gpsimd.memset(ones16[:, :], 1.0)
    ident = consts.tile([128, 128], f16)
    nc.gpsimd.affine_select(out=ident[:, :], in_=ones16[:, :],
                            pattern=[[1, 128]],
                            compare_op=ALU.is_equal, fill=0.0,
                            base=0, channel_multiplier=-1)

    # warm the ACT Sin table immediately (single-dep activation keeps the
    # auto-inserted LoadActFuncSet ahead of any multi-wait semaphore bundle)
    warm = consts.tile([128, 1], f32)
    nc.scalar.activation(warm[:, :], bias_c[:, 0:1], AF.Sin,
                         bias=0.0, scale=1.0)

    # vcat tiles + zero their pad slots early (pads flow into the transposes)
    vch = []
    for h in (0, 1):
        vc = sb.tile([128, 256], f16, name=f"vc{h}")
        E = nc.vector if h == 0 else nc.gpsimd
        E.memset(vc[:, 240:256], 0.0)
        vch.append(vc)

    # ---- trig: cs[40h + 20t + 4j + g]: t=0: cos(x_j/2)/2 = c4^2 - 1/2,
    # t=1: sin(x_j/2) DIRECT from the table (|x/2| < pi; the mixed per-slot
    # scaling is folded into mproj on the host). c4 = cos(x/4) first.
    sc4 = sb.tile([128, 40], f16)
    cs = sb.tile([128, 80], f16)
    nc.scalar.activation(fv(sc4, 0, [[20, 2], [4, 5], [1, 4]]),
                         fv(x_s, 0, [[48, 2], [1, 5], [12, 4]]),
                         AF.Sin, bias=bias_c[:, 0:1], scale=0.25)
    nc.scalar.activation(fv(cs, 20, [[40, 2], [4, 5], [1, 4]]),
                         fv(x_s, 0, [[48, 2], [1, 5], [12, 4]]),
                         AF.Sin, bias=0.0, scale=0.5)

    # ---- per-half slot assembly + prefix kron tree into vcat (all packed)
    from concourse.bass import _add_dep_helper
    kron_last = {}
    for h in (0, 1):
        E = nc.vector if h == 0 else nc.gpsimd
        sqt = sb.tile([128, 20], f16, name=f"sq{h}")
        ctmp = sb.tile([128, 16], f16, name=f"ct{h}")
        vc = vch[h]

        class _CView:
            """cs columns for this half (offset 40h)."""
            tensor = cs.tensor
            offset = cs.offset + 40 * h
            ap = cs.ap
        cst = _CView
        # sq = c4^2 ; cs_c = sq - 1/2  (cs_s came straight from ACT)
        E.tensor_mul(fv(sqt, 0, [[4, 5], [1, 4]]),
                     fv(sc4, 20 * h, [[4, 5], [1, 4]]),
                     fv(sc4, 20 * h, [[4, 5], [1, 4]]))
        E.tensor_scalar_sub(fv(cst, 0, [[4, 5], [1, 4]]),
                            fv(sqt, 0, [[4, 5], [1, 4]]), 0.5)
        # u_j[t, g] at cst col 20t + 4j + g
        # A[a=(z0 z1)] -> vcat slots 56..60 (cols 224..240)
        E.tensor_mul(fv(vc, 224, [[8, 2], [4, 2], [1, 4]]),
                     fv(cst, 4, [[0, 2], [20, 2], [1, 4]]),
                     fv(cst, 0, [[20, 2], [0, 2], [1, 4]]))
        # C[(z3 z4)] -> ctmp
        E.tensor_mul(fv(ctmp, 0, [[8, 2], [4, 2], [1, 4]]),
                     fv(cst, 16, [[0, 2], [20, 2], [1, 4]]),
                     fv(cst, 12, [[20, 2], [0, 2], [1, 4]]))
        # B[(a z2)] -> slots 48..56 (cols 192..224)
        E.tensor_mul(fv(vc, 192, [[8, 4], [4, 2], [1, 4]]),
                     fv(vc, 224, [[4, 4], [0, 2], [1, 4]]),
                     fv(cst, 8, [[0, 4], [20, 2], [1, 4]]))
        # R[(b c2)] -> slots 0..32 (cols 0..128)   (chunk0 = pure R)
        # the big R op runs on DVE for BOTH halves (Pool is 2.4x slower and
        # R gates the chunk0 transpose); D stays on E (parallel, chunk1 only)
        r_i = nc.vector.tensor_mul(fv(vc, 0, [[16, 8], [4, 4], [1, 4]]),
                                   fv(vc, 192, [[4, 8], [0, 4], [1, 4]]),
                                   fv(ctmp, 0, [[0, 8], [4, 4], [1, 4]]))
        # D[(b z3)] -> slots 32..48 (cols 128..192)  (feeds only chunk1)
        d_i = E.tensor_mul(fv(vc, 128, [[8, 8], [4, 2], [1, 4]]),
                           fv(vc, 192, [[4, 8], [0, 2], [1, 4]]),
                           fv(cst, 12, [[0, 8], [20, 2], [1, 4]]))
        kron_last[h] = (r_i, d_i)
    # keep h1's DVE-side R op behind h0's whole kron in the DVE queue
    # (otherwise its Pool dependency head-of-line blocks the h0 chain)
    _add_dep_helper(kron_last[1][0].ins, kron_last[0][1].ins, sync=False,
                    reason="R1 after kron0")

    # ---- all four transposes first (explicit PE order: T's ahead of YT
    # matmuls, so no YT blocks a later transpose's dependents)
    tps = {}
    t_insts = []
    for h in (0, 1):
        for c in (0, 1):
            tp = ps.tile([128, 128], f16, tag="tp", bufs=4)
            ti = nc.tensor.transpose(tp[:, :],
                                     vch[h][:, 128 * c:128 * c + 128],
                                     ident[:, :])
            tps[(h, c)] = tp
            t_insts.append(ti)

    y_s = sb.tile([128, 96], f32)
    for h in (0, 1):
        # both chunks' transposed features in ONE tile -> one merged pm op
        vs = sb2.tile([128, 256], f16, tag=f"vT{h}", bufs=1)
        nc.vector.tensor_copy(vs[:, 0:128], tps[(h, 0)][:, :])
        nc.vector.tensor_copy(vs[:, 128:256], tps[(h, 1)][:, :])
        # YT = mproj^T @ vcatT (per-half PSUM bank holds both chunks; mp1 is
        # zero-padded to 128 cols so the whole bank is written). The y4
        # result also lives in this bank (cols 256:304) — the bank is dead
        # after the pm mul, and per-half separation avoids cross-half
        # tile-granular WAR stalls.
        ytb = ps.tile([128, 304], f32, tag=f"YT{h}", bufs=1)
        yt_i = nc.tensor.matmul(ytb[:, 0:128], lhsT=mp0_v, rhs=vs[:, 0:128],
                                start=True, stop=True)
        if h == 0:
            # keep the last transpose ahead of the first YT in the PE queue
            _add_dep_helper(yt_i.ins, t_insts[-1].ins, sync=False,
                            reason="T before YT")
        nc.tensor.matmul(ytb[:, 128:256], lhsT=mp1_v, rhs=vs[:, 128:256],
                         start=True, stop=True)
        # PmT = YT * vcatT: ONE merged elementwise op per half (DVE)
        pm = sb2.tile([128, 256], f16, tag=f"Pm{h}", bufs=1)
        pm_i = nc.vector.tensor_mul(pm[:, :], ytb[:, 0:256], vs[:, :])
        if h == 0:
            pm_h0 = pm_i
        else:
            # h1's PmT mul behind h0's in the DVE queue
            _add_dep_helper(pm_i.ins, pm_h0.ins, sync=False,
                            reason="pm order")
        # hT = W1X0^T @ PmT0 + W1X1^T @ PmT1 (reduction + W1 in one step)
        hT_p = ps.tile([128, 128], f32, tag=f"hT{h}", bufs=1)
        nc.tensor.matmul(hT_p[:, :], lhsT=w1x0_v, rhs=pm[:, 0:128],
                         start=True, stop=False)
        nc.tensor.matmul(hT_p[:, :], lhsT=w1x1_v, rhs=pm[:, 128:256],
                         start=False, stop=True)
        # relu with b1 folded in: h0 on ACT, h1 on DVE (free after the pm
        # muls; saves the ACT 185-cycle access on the critical h1 lane)
        hT_s = sb2.tile([128, 128], f16, tag=f"hTs{h}", bufs=1)
        if h == 0:
            nc.scalar.activation(hT_s[:, :], hT_p[:, :], AF.Relu,
                                 bias=b1_v, scale=1.0)
        else:
            r1_i = nc.vector.tensor_scalar(hT_s[:, :], hT_p[:, :], b1_v,
                                           0.0, ALU.add, ALU.max)
            _add_dep_helper(r1_i.ins, pm_i.ins, sync=False,
                            reason="relu1 after pm1")
        nc.tensor.matmul(ytb[:, 256:304], lhsT=hT_s[:, :],
                         rhs=w2_v, start=True, stop=True)
        # b2 add doubles as the PSUM->SBUF copy
        nc.vector.tensor_add(fv(y_s, 48 * h, [[1, 48]]),
                             fv(ytb, 256, [[1, 48]]), b2_v)
        ya = bass.AP(tensor=y.tensor, offset=48 * h, ap=[[96, 128], [1, 48]])
        nc.sync.dma_start(ya, y_s[:, 48 * h:48 * h + 48])

    return xdma


def _hoist_pre_barrier(nc, inst):
    """Move `inst` (a BassInstruction) into the entry block before the first
    SP-engine instruction (i.e. before the all-engine start barrier)."""
    from concourse import mybir
    ins = inst.ins
    fn = nc.m.functions[0]
    blocks = fn.blocks
    src = None
    for b in blocks:
        for i2 in b.instructions:
            if i2.name == ins.name:
                src = b
                break
        if src is not None:
            break
    assert src is not None, "hoist: dma instruction not found"
    entry = blocks[0]
    src.instructions.remove(ins)
    idx = 0
    for k, i2 in enumerate(entry.instructions):
        if i2.engine == mybir.EngineType.SP:
            idx = k
            break
    entry.instructions.insert(idx, ins)


_NC_CACHE = {}


def _get_nc():
    if "nc" in _NC_CACHE:
        return _NC_CACHE["nc"]
    from contextlib import ExitStack
    import concourse.bacc as bacc
    import concourse.tile as tile
    from concourse import mybir
    f32 = mybir.dt.float32
    f16 = mybir.dt.float16
    nc = bacc.Bacc("TRN2", target_bir_lowering=False, debug=False)
    x = nc.dram_tensor("x", [BLOC, 12], f16, kind="ExternalInput").ap()
    hcst = nc.dram_tensor("hcst", [128, 560], f16, kind="ExternalInput").ap()
    fcst = nc.dram_tensor("fcst", [128, 52], f32, kind="ExternalInput").ap()
    y = nc.dram_tensor("y", [BLOC, 12], f32, kind="ExternalOutput").ap()
    with tile.TileContext(nc) as tc:
        with ExitStack() as ctx:
            xdma = _build_body(ctx, tc, x, hcst, fcst, y)
    _hoist_pre_barrier(nc, xdma)
    nc.compile()
    _NC_CACHE["nc"] = nc
    return nc


def _run(inputs_np, consts, trace=False):
    from concourse.bass_utils import run_bass_kernel_spmd
    nc = _get_nc()
    x = np.ascontiguousarray(np.asarray(inputs_np, np.float32).astype(np.float16))
    in_maps = []
    for c in range(NCORES):
        m = {"x": np.ascontiguousarray(x[BLOC * c:BLOC * (c + 1)])}
        m.update(consts)
        in_maps.append(m)
    res = run_bass_kernel_spmd(nc, in_maps, core_ids=list(range(NCORES)),
                               trace=trace)
    out = np.concatenate([r["y"] for r in res.results], axis=0)
    return out.astype(np.float32), res


def kernel(inputs, q_params, W1, b1, W2, b2):
    consts = _host_consts(q_params, W1, b1, W2, b2)
    out, _ = _run(inputs, consts, trace=False)
    return out
